# revision 14
# baseline (speedup 1.0000x reference)
"""Mixtral MoE layer (top-2 of 8 experts) as a Trainium2 Bass/Tile kernel.

Strategy (expert-parallel, per the original tp_size/expert_indices code):
  - 8 NeuronCores, one expert per core. Sparse dispatch: the host replays the
    router (numpy) to decide token->core sharding AND to compute the final
    per-token routing weights; each core receives only the ~T/4 tokens routed
    to its expert (padded to a static 64-aligned capacity), its token scales,
    and its expert's weights. No routing runs on device at all.
  - Host pre-transposes x -> xT [H,C] and weights -> w1t/w3t [H,F], w2t [F,H]
    so every SBUF tile load is a natural contiguous 2D slice. x and weights
    are cast to bf16 on the host; all accumulation is fp32 in PSUM.
  - On device, each core runs its expert MLP in transposed space: h1T/h3T
    [F-part, tok-col] = w1t/w3t-tile.T @ xT, g = silu(h1T)*h3T (bf16),
    out [tok-part, H-col] = gT-tile.T @ w2t, then scales rows by the
    host-provided routing weight (a per-partition scalar) and writes a [C,H]
    partial. The host scatter-adds partials back to [T,H] ("all-reduce").
  - Startup is tuned against the PE HAM clock gate (1.2 -> 2.4 GHz): a short
    dummy-matmul warmup covers the ~6.5us framework preamble, x tiles stream
    on the scalar-engine HWDGE queue in parallel with weights on the sync
    queue, and the first quarter's w1/w3 loads are split into small head
    tiles so real GEMMs start as soon as data lands and the PE never idles
    (an idle window mid-ramp re-throttles the clock for ~40us).

Tiling: single token chunk of TC; F processed in NQ=4 quarters with an SBUF
fp32 accumulator for GEMM3 partials; moving-operand slices capped at 512;
token tiles of 128 with an optional 64 tail. Dense fallback (all tokens on
every core) guards pathological routing imbalance (capacity > 2048).
"""

from contextlib import ExitStack

import ml_dtypes
import numpy as np

import concourse.bacc as bacc
import concourse.tile as tile
from concourse import mybir
from concourse.bass_utils import run_bass_kernel_spmd

P = 128
AF = mybir.ActivationFunctionType
OP = mybir.AluOpType
AX = mybir.AxisListType
F32 = mybir.dt.float32
BF16 = mybir.dt.bfloat16


def build_moe_nc(T=1088, H=1024, F=3584, TC=1088, NQ=4, WU=11, silu_native=True):
    """Build the single-core SPMD program. Returns the compiled Bacc."""
    HT = H // P          # contraction k-tiles for GEMM1/2
    FT = F // P          # f tiles
    FQ = FT // NQ        # f-tiles per quarter
    NCH = T // TC        # token chunks
    # moving-operand slices of the token chunk (max free dim 512). Balanced
    # widths (e.g. 384/384/320 rather than 512/512/64): a tiny tail slice is
    # NX-issue-floor-bound (~56ns) while balanced slices stay stream-bound.
    nsl = -(-TC // 512)
    k64 = TC // 64
    nw_slices = []
    off = 0
    for i in range(nsl):
        w = (k64 // nsl + (1 if i < k64 % nsl else 0)) * 64
        nw_slices.append((off, w))
        off += w
    assert off == TC
    # 128-token tiles per chunk, with an optional 64-token tail tile
    ttiles = []
    toff = 0
    while toff < TC:
        th_ = min(P, TC - toff)
        ttiles.append((toff, th_))
        toff += th_
    NTT = len(ttiles)
    HW = min(512, H)     # output H column group width
    HC = H // HW
    NSC = NTT * NCH      # scale columns (one per token tile)
    FA = min(2, FQ)      # f-tiles in the quarter-0 "head" weight load
    assert FT % NQ == 0 and T % TC == 0 and TC % 64 == 0 and H % HW == 0

    nc = bacc.Bacc("TRN2", target_bir_lowering=False, debug=False)
    xt_d = nc.dram_tensor("xt", [H, T], BF16, kind="ExternalInput").ap()
    sc_d = nc.dram_tensor("sc", [P, NSC], F32, kind="ExternalInput").ap()
    w1t_d = nc.dram_tensor("w1t", [H, F], BF16, kind="ExternalInput").ap()
    w3t_d = nc.dram_tensor("w3t", [H, F], BF16, kind="ExternalInput").ap()
    w2t_d = nc.dram_tensor("w2t", [F, H], BF16, kind="ExternalInput").ap()
    out_d = nc.dram_tensor("out", [T, H], BF16, kind="ExternalOutput").ap()

    with tile.TileContext(nc) as tc, ExitStack() as ctx:
        xt_pool = ctx.enter_context(tc.tile_pool(name="xt", bufs=HT))
        wa_pool = ctx.enter_context(tc.tile_pool(name="wa", bufs=2 * HT))
        wb_pool = ctx.enter_context(tc.tile_pool(name="wb", bufs=2 * HT))
        w13_pool = ctx.enter_context(tc.tile_pool(name="w13", bufs=2 * HT))
        w2_pool = ctx.enter_context(tc.tile_pool(name="w2", bufs=FQ))
        g_pool = ctx.enter_context(tc.tile_pool(name="g", bufs=FQ))
        acc_pool = ctx.enter_context(tc.tile_pool(name="acc", bufs=NTT * HC))
        tmp_pool = ctx.enter_context(tc.tile_pool(name="tmp", bufs=4))
        sc_pool = ctx.enter_context(tc.tile_pool(name="scp", bufs=1))
        ob_pool = ctx.enter_context(tc.tile_pool(name="ob", bufs=4))
        ps12 = ctx.enter_context(tc.tile_pool(name="ps12", bufs=6, space="PSUM"))
        ps3 = ctx.enter_context(tc.tile_pool(name="ps3", bufs=2, space="PSUM"))

        # ---- PE warm-up: dummy matmuls covering the first DMA landings
        # (user code on every engine starts only after the ~6.3us framework
        # preamble barrier), so the HAM clock gate sees sustained activity
        # and real GEMMs take over at full clock without a gap.
        wu_w = tmp_pool.tile([P, P], BF16, tag="wu")
        wu_x = tmp_pool.tile([P, 512], BF16, tag="wu2")
        nc.vector.memset(wu_w[:], 0.0)
        nc.vector.memset(wu_x[:], 0.0)
        wu_ps = ps3.tile([P, 512], F32, tag="ps3")
        for i in range(WU):
            nc.tensor.matmul(wu_ps[:], wu_w[:], wu_x[:],
                             start=(i == 0), stop=(i == WU - 1))

        # routing scales (host-computed): one tiny DMA, first on sync queue
        sct = sc_pool.tile([P, NSC], F32, tag="sc")
        nc.sync.dma_start(out=sct[:], in_=sc_d[:, :])

        for c in range(NCH):
            t0 = c * TC
            # ---- xT chunk [H, TC] bf16 tiles; loads alternate between the
            # two HWDGE queues (scalar/sync) so triggers and transfers of
            # consecutive h-tiles overlap
            xts = []
            for h in range(HT):
                t_ = xt_pool.tile([P, TC], BF16, tag="xt")
                eng = nc.scalar if h % 2 == 0 else nc.sync
                eng.dma_start(out=t_[:], in_=xt_d[h * P:(h + 1) * P, t0:t0 + TC])
                xts.append(t_)
                if c == 0:
                    # ramp: interleave the quarter-0 head weights per h on
                    # the same queue as x[h] so each h's (x, w1a, w3a) trio
                    # lands together, in consumption order
                    ta1 = wa_pool.tile([P, FA * P], BF16, tag="w1a")
                    eng.dma_start(out=ta1[:], in_=w1t_d[h * P:(h + 1) * P, 0:FA * P])
                    ta3 = wa_pool.tile([P, FA * P], BF16, tag="w3a")
                    eng.dma_start(out=ta3[:], in_=w3t_d[h * P:(h + 1) * P, 0:FA * P])
                    if h == 0:
                        w1h, w3h = [], []
                    w1h.append(ta1)
                    w3h.append(ta3)

            acc_tiles = {}
            for q in range(NQ):
                f0 = q * FQ * P
                # ---- quarter weight loads. Quarter 0 is split per h into a
                # small head (first FA f-tiles) + remainder so the very first
                # GEMMs are gated on ~65KB tiles, not a whole 1.8MB quarter.
                if q == 0 and c == 0:
                    w1r, w3r = [], []
                    for h in range(HT):
                        eng = nc.sync
                        tb1 = wb_pool.tile([P, (FQ - FA) * P], BF16, tag="w1b")
                        eng.dma_start(
                            out=tb1[:],
                            in_=w1t_d[h * P:(h + 1) * P, f0 + FA * P:f0 + FQ * P])
                        w1r.append(tb1)
                        tb3 = wb_pool.tile([P, (FQ - FA) * P], BF16, tag="w3b")
                        eng.dma_start(
                            out=tb3[:],
                            in_=w3t_d[h * P:(h + 1) * P, f0 + FA * P:f0 + FQ * P])
                        w3r.append(tb3)

                    def lw(fq, h, which,
                           _a1=w1h, _a3=w3h, _b1=w1r, _b3=w3r, _fa=FA):
                        if fq < _fa:
                            tl = (_a1 if which == 1 else _a3)[h]
                            return tl[:, fq * P:(fq + 1) * P]
                        tl = (_b1 if which == 1 else _b3)[h]
                        return tl[:, (fq - _fa) * P:(fq - _fa + 1) * P]
                else:
                    w1q, w3q = [], []
                    for h in range(HT):
                        t1 = w13_pool.tile([P, FQ * P], BF16, tag="w13")
                        nc.sync.dma_start(
                            out=t1[:], in_=w1t_d[h * P:(h + 1) * P, f0:f0 + FQ * P])
                        w1q.append(t1)
                        t3 = w13_pool.tile([P, FQ * P], BF16, tag="w13")
                        nc.sync.dma_start(
                            out=t3[:], in_=w3t_d[h * P:(h + 1) * P, f0:f0 + FQ * P])
                        w3q.append(t3)

                    def lw(fq, h, which, _w1q=w1q, _w3q=w3q):
                        tl = (_w1q if which == 1 else _w3q)[h]
                        return tl[:, fq * P:(fq + 1) * P]
                w2q = []
                for fq in range(FQ):
                    f = q * FQ + fq
                    t2 = w2_pool.tile([P, H], BF16, tag="w2")
                    nc.sync.dma_start(out=t2[:], in_=w2t_d[f * P:(f + 1) * P, :])
                    w2q.append(t2)

                # ---- GEMM1/2: h1T/h3T [P(F), NW] + silu*mul -> g tiles
                # [P, TC]. Emitted h-major with p1/p3 interleaved per h so
                # compute tracks per-h DMA arrival during the ramp.
                gq = []
                for fq in range(FQ):
                    p1 = [ps12.tile([P, w], F32, tag="ps12", name=f"p1_{c}_{q}_{fq}_{th}")
                          for th, (o, w) in enumerate(nw_slices)]
                    p3 = [ps12.tile([P, w], F32, tag="ps12", name=f"p3_{c}_{q}_{fq}_{th}")
                          for th, (o, w) in enumerate(nw_slices)]
                    for h in range(HT):
                        l1 = lw(fq, h, 1)
                        for th, (o, w) in enumerate(nw_slices):
                            nc.tensor.matmul(
                                p1[th][:], l1, xts[h][:, o:o + w],
                                start=(h == 0), stop=(h == HT - 1))
                        l3 = lw(fq, h, 3)
                        for th, (o, w) in enumerate(nw_slices):
                            nc.tensor.matmul(
                                p3[th][:], l3, xts[h][:, o:o + w],
                                start=(h == 0), stop=(h == HT - 1))
                    gt = g_pool.tile([P, TC], BF16, tag="g")
                    for th, (o, w) in enumerate(nw_slices):
                        tmp = tmp_pool.tile([P, w], F32, tag="tmp")
                        if silu_native:
                            nc.scalar.activation(tmp[:], p1[th][:], AF.Silu)
                        else:
                            # CoreSim has no Silu; sigmoid then explicit mul
                            sg = tmp_pool.tile([P, w], F32, tag="tmp")
                            nc.scalar.activation(sg[:], p1[th][:], AF.Sigmoid)
                            nc.vector.tensor_tensor(tmp[:], sg[:], p1[th][:], OP.mult)
                        nc.vector.tensor_tensor(
                            gt[:, o:o + w], tmp[:], p3[th][:], OP.mult)
                    gq.append(gt)

                # ---- GEMM3: out[T-part, H-col] partial over this quarter's F
                for tt, (to, th_) in enumerate(ttiles):
                    for hcol in range(HC):
                        po = ps3.tile([th_, HW], F32, tag="ps3")
                        for fq in range(FQ):
                            nc.tensor.matmul(
                                po[:], gq[fq][:, to:to + th_],
                                w2q[fq][:, hcol * HW:(hcol + 1) * HW],
                                start=(fq == 0), stop=(fq == FQ - 1))
                        scl = sct[0:th_, c * NTT + tt:c * NTT + tt + 1]
                        if q == 0:
                            at = acc_pool.tile([th_, HW], F32, tag="acc")
                            acc_tiles[(tt, hcol)] = at
                            if NQ == 1:
                                ob = ob_pool.tile([th_, HW], BF16, tag="ob")
                                nc.vector.tensor_scalar(
                                    ob[:], po[:], scl, None, OP.mult)
                                nc.sync.dma_start(
                                    out=out_d[t0 + to:t0 + to + th_,
                                              hcol * HW:(hcol + 1) * HW],
                                    in_=ob[:])
                            else:
                                nc.scalar.copy(at[:], po[:])
                        else:
                            at = acc_tiles[(tt, hcol)]
                            nc.vector.tensor_tensor(at[:], po[:], at[:], OP.add)
                            if q == NQ - 1:
                                ob = ob_pool.tile([th_, HW], BF16, tag="ob")
                                nc.vector.tensor_scalar(
                                    ob[:], at[:], scl, None, OP.mult)
                                nc.sync.dma_start(
                                    out=out_d[t0 + to:t0 + to + th_,
                                              hcol * HW:(hcol + 1) * HW],
                                    in_=ob[:])

    nc.compile()
    return nc


def _routing(x2, gate_w):
    """Host replay of the router: token index list + renormalized top-2
    weight per (expert, token). Selection uses the same fp32 logits /
    stable argsort as the reference's top_k; weights computed in fp64."""
    logits = x2.astype(np.float32) @ gate_w.astype(np.float32).T
    order = np.argsort(-logits, axis=1, kind="stable")[:, :2]
    E = gate_w.shape[0]
    idx = [np.nonzero((order == e).any(axis=1))[0] for e in range(E)]
    l64 = logits.astype(np.float64)
    l64 -= l64.max(axis=1, keepdims=True)
    ex = np.exp(l64)
    p = ex / ex.sum(axis=1, keepdims=True)
    t = np.arange(p.shape[0])
    denom = p[t, order[:, 0]] + p[t, order[:, 1]]
    return idx, p, denom


def _host_top2_idx(x2, gate_w):
    """Token index list per expert (host copy of the routing, for sharding)."""
    return _routing(x2, gate_w)[0]


def _sc_plane(weights, cpad, ntt):
    """Pack per-token scales into the [128, NSC] plane the kernel expects."""
    flat = np.zeros(ntt * P, dtype=np.float32)
    flat[:len(weights)] = weights
    assert ntt * P >= cpad
    return np.ascontiguousarray(flat.reshape(ntt, P).T)


_NC_CACHE = {}


def _get_nc(key, **kw):
    if key not in _NC_CACHE:
        _NC_CACHE[key] = build_moe_nc(**kw)
    return _NC_CACHE[key]


def kernel(hidden_states, gate_w, w1, w2, w3, _trace=False, _trace_kwargs=None):
    B, S, H = hidden_states.shape
    E = gate_w.shape[0]
    T = B * S
    x2 = np.asarray(hidden_states, dtype=np.float32).reshape(T, H)
    idx, p, denom = _routing(x2, gate_w)
    tarange = np.arange(T)
    cmax = max(len(i) for i in idx)
    cpad = max(512, -(-cmax // 64) * 64)
    xt16 = np.ascontiguousarray(x2.T).astype(ml_dtypes.bfloat16)
    wdt = ml_dtypes.bfloat16

    def expert_weights(e):
        return {
            "w1t": np.ascontiguousarray(
                np.asarray(w1[e], dtype=np.float32).T).astype(wdt),
            "w3t": np.ascontiguousarray(
                np.asarray(w3[e], dtype=np.float32).T).astype(wdt),
            "w2t": np.ascontiguousarray(
                np.asarray(w2[e], dtype=np.float32).T).astype(wdt),
        }

    if cpad <= 2048:
        # sparse path: each core gets only its expert's tokens (padded)
        ntt = -(-cpad // P)
        nc = _get_nc(("sparse", cpad), T=cpad, TC=cpad, NQ=4)
        in_maps = []
        for e in range(E):
            xg = np.zeros((H, cpad), dtype=ml_dtypes.bfloat16)
            xg[:, :len(idx[e])] = xt16[:, idx[e]]
            w_e = (p[idx[e], e] / denom[idx[e]]).astype(np.float32)
            m = expert_weights(e)
            m["xt"] = xg
            m["sc"] = _sc_plane(w_e, cpad, ntt)
            in_maps.append(m)
        res = run_bass_kernel_spmd(
            nc, in_maps, list(range(E)), trace=_trace, **(_trace_kwargs or {}))
        kernel.last_results = res
        out = np.zeros((T, H), dtype=np.float32)
        for e, r in enumerate(res.results):
            out[idx[e]] += r["out"][:len(idx[e])].astype(np.float32)
    else:
        # dense fallback (pathological routing imbalance): every core runs
        # all tokens for its expert; non-selected tokens get scale 0.
        TC = 1024
        ntt = -(-TC // P) * (T // TC)
        nc = _get_nc(("dense", T), T=T, TC=TC, NQ=4)
        in_maps = []
        for e in range(E):
            w_e = np.zeros(T, dtype=np.float32)
            w_e[idx[e]] = (p[idx[e], e] / denom[idx[e]]).astype(np.float32)
            m = expert_weights(e)
            m["xt"] = xt16
            m["sc"] = _sc_plane(w_e, T, ntt)
            in_maps.append(m)
        res = run_bass_kernel_spmd(
            nc, in_maps, list(range(E)), trace=_trace, **(_trace_kwargs or {}))
        kernel.last_results = res
        out = np.zeros((T, H), dtype=np.float32)
        for r in res.results:
            out += r["out"].astype(np.float32)
    return out.reshape(B, S, H).astype(hidden_states.dtype)


# revision 22
# speedup vs baseline: 1.0163x; 1.0163x over previous
"""Mixtral MoE layer (top-2 of 8 experts) as a Trainium2 Bass/Tile kernel.

Strategy (expert-parallel, per the original tp_size/expert_indices code):
  - 8 NeuronCores, one expert per core. Sparse dispatch: the host replays the
    router (numpy) to decide token->core sharding AND to compute the final
    per-token routing weights; each core receives only the ~T/4 tokens routed
    to its expert (padded to a static 64-aligned capacity), its token scales,
    and its expert's weights. No routing runs on device at all.
  - Host pre-transposes x -> xT [H,C] and weights -> w1t/w3t [H,F], w2t [F,H]
    so every SBUF tile load is a natural contiguous 2D slice. x and weights
    are cast to bf16 on the host; all accumulation is fp32 in PSUM.
  - On device, each core runs its expert MLP in transposed space: h1T/h3T
    [F-part, tok-col] = w1t/w3t-tile.T @ xT, g = silu(h1T)*h3T (bf16),
    out [tok-part, H-col] = gT-tile.T @ w2t, then scales rows by the
    host-provided routing weight (a per-partition scalar) and writes a [C,H]
    partial. The host scatter-adds partials back to [T,H] ("all-reduce").
  - Startup is tuned against the PE HAM clock gate (1.2 -> 2.4 GHz): a short
    dummy-matmul warmup covers the ~6.5us framework preamble, x tiles stream
    on the scalar-engine HWDGE queue in parallel with weights on the sync
    queue, and the first quarter's w1/w3 loads are split into small head
    tiles so real GEMMs start as soon as data lands and the PE never idles
    (an idle window mid-ramp re-throttles the clock for ~40us).

Tiling: single token chunk of TC; F processed in NQ=4 quarters with an SBUF
fp32 accumulator for GEMM3 partials; moving-operand slices capped at 512;
token tiles of 128 with an optional 64 tail. Dense fallback (all tokens on
every core) guards pathological routing imbalance (capacity > 2048).
"""

from contextlib import ExitStack

import ml_dtypes
import numpy as np

import concourse.bacc as bacc
import concourse.tile as tile
from concourse import mybir
from concourse.bass_utils import run_bass_kernel_spmd

P = 128
AF = mybir.ActivationFunctionType
OP = mybir.AluOpType
AX = mybir.AxisListType
F32 = mybir.dt.float32
BF16 = mybir.dt.bfloat16


def build_moe_nc(T=1088, H=1024, F=3584, TC=1088, NQ=4, WU=12, silu_native=True):
    """Build the single-core SPMD program. Returns the compiled Bacc."""
    HT = H // P          # contraction k-tiles for GEMM1/2
    FT = F // P          # f tiles
    FQ = FT // NQ        # f-tiles per quarter
    NCH = T // TC        # token chunks
    # moving-operand slices of the token chunk (max free dim 512). Balanced
    # widths (e.g. 384/384/320 rather than 512/512/64): a tiny tail slice is
    # NX-issue-floor-bound (~56ns) while balanced slices stay stream-bound.
    nsl = -(-TC // 512)
    k64 = TC // 64
    nw_slices = []
    off = 0
    for i in range(nsl):
        w = (k64 // nsl + (1 if i < k64 % nsl else 0)) * 64
        nw_slices.append((off, w))
        off += w
    assert off == TC
    # 128-token tiles per chunk, with an optional 64-token tail tile
    ttiles = []
    toff = 0
    while toff < TC:
        th_ = min(P, TC - toff)
        ttiles.append((toff, th_))
        toff += th_
    NTT = len(ttiles)
    HW = min(512, H)     # output H column group width
    HC = H // HW
    NSC = NTT * NCH      # scale columns (one per token tile)
    FA = min(2, FQ)      # f-tiles in the quarter-0 "head" weight load
    assert FT % NQ == 0 and T % TC == 0 and TC % 64 == 0 and H % HW == 0

    nc = bacc.Bacc("TRN2", target_bir_lowering=False, debug=False)
    xt_d = nc.dram_tensor("xt", [H, T], BF16, kind="ExternalInput").ap()
    w1t_d = nc.dram_tensor("w1t", [H, F], BF16, kind="ExternalInput").ap()
    w3t_d = nc.dram_tensor("w3t", [H, F], BF16, kind="ExternalInput").ap()
    w2t_d = nc.dram_tensor("w2t", [F, H], BF16, kind="ExternalInput").ap()
    # transposed output [H, T]: GEMM3 runs with w2 stationary / g moving so
    # tokens stream at their natural count (no 128-padded tail tile); the
    # host applies the routing scale during the scatter-add.
    out_d = nc.dram_tensor("out", [H, T], BF16, kind="ExternalOutput").ap()

    with tile.TileContext(nc) as tc, ExitStack() as ctx:
        xt_pool = ctx.enter_context(tc.tile_pool(name="xt", bufs=HT))
        wa_pool = ctx.enter_context(tc.tile_pool(name="wa", bufs=2 * HT))
        wb_pool = ctx.enter_context(tc.tile_pool(name="wb", bufs=2 * HT))
        w13_pool = ctx.enter_context(tc.tile_pool(name="w13", bufs=2 * HT))
        w2_pool = ctx.enter_context(tc.tile_pool(name="w2", bufs=FQ))
        g_pool = ctx.enter_context(tc.tile_pool(name="g", bufs=FQ))
        acc_pool = ctx.enter_context(tc.tile_pool(name="acc", bufs=HT))
        tmp_pool = ctx.enter_context(tc.tile_pool(name="tmp", bufs=4))
        ob_pool = ctx.enter_context(tc.tile_pool(name="ob", bufs=4))
        ps12 = ctx.enter_context(tc.tile_pool(name="ps12", bufs=6, space="PSUM"))
        ps3 = ctx.enter_context(tc.tile_pool(name="ps3", bufs=2, space="PSUM"))

        # ---- PE warm-up: dummy matmuls covering the first DMA landings
        # (user code on every engine starts only after the ~6.3us framework
        # preamble barrier), so the HAM clock gate sees sustained activity
        # and real GEMMs take over at full clock without a gap.
        wu_w = tmp_pool.tile([P, P], BF16, tag="wu")
        wu_x = tmp_pool.tile([P, 512], BF16, tag="wu2")
        nc.vector.memset(wu_w[:], 0.0)
        nc.vector.memset(wu_x[:], 0.0)
        wu_ps = ps3.tile([P, 512], F32, tag="ps3")
        for i in range(WU):
            nc.tensor.matmul(wu_ps[:], wu_w[:], wu_x[:],
                             start=(i == 0), stop=(i == WU - 1))

        for c in range(NCH):
            t0 = c * TC
            # ---- xT chunk [H, TC] bf16 tiles; loads alternate between the
            # two HWDGE queues (scalar/sync) so triggers and transfers of
            # consecutive h-tiles overlap
            xts = []
            for h in range(HT):
                t_ = xt_pool.tile([P, TC], BF16, tag="xt")
                # sync queue's user stream starts ~3us before scalar's (act
                # tables load first there), so even trios go on sync
                eng = nc.sync if h % 2 == 0 else nc.scalar
                eng.dma_start(out=t_[:], in_=xt_d[h * P:(h + 1) * P, t0:t0 + TC])
                xts.append(t_)
                if c == 0:
                    # ramp: interleave the quarter-0 head weights per h on
                    # the same queue as x[h] so each h's (x, w1a, w3a) trio
                    # lands together, in consumption order
                    ta1 = wa_pool.tile([P, FA * P], BF16, tag="w1a")
                    eng.dma_start(out=ta1[:], in_=w1t_d[h * P:(h + 1) * P, 0:FA * P])
                    ta3 = wa_pool.tile([P, FA * P], BF16, tag="w3a")
                    eng.dma_start(out=ta3[:], in_=w3t_d[h * P:(h + 1) * P, 0:FA * P])
                    if h == 0:
                        w1h, w3h = [], []
                    w1h.append(ta1)
                    w3h.append(ta3)

            acc_tiles = {}
            for q in range(NQ):
                f0 = q * FQ * P
                # ---- quarter weight loads. Quarter 0 is split per h into a
                # small head (first FA f-tiles) + remainder so the very first
                # GEMMs are gated on ~65KB tiles, not a whole 1.8MB quarter.
                if q == 0 and c == 0:
                    w1r, w3r = [], []
                    for h in range(HT):
                        eng = nc.sync
                        tb1 = wb_pool.tile([P, (FQ - FA) * P], BF16, tag="w1b")
                        eng.dma_start(
                            out=tb1[:],
                            in_=w1t_d[h * P:(h + 1) * P, f0 + FA * P:f0 + FQ * P])
                        w1r.append(tb1)
                        tb3 = wb_pool.tile([P, (FQ - FA) * P], BF16, tag="w3b")
                        eng.dma_start(
                            out=tb3[:],
                            in_=w3t_d[h * P:(h + 1) * P, f0 + FA * P:f0 + FQ * P])
                        w3r.append(tb3)

                    def lw(fq, h, which,
                           _a1=w1h, _a3=w3h, _b1=w1r, _b3=w3r, _fa=FA):
                        if fq < _fa:
                            tl = (_a1 if which == 1 else _a3)[h]
                            return tl[:, fq * P:(fq + 1) * P]
                        tl = (_b1 if which == 1 else _b3)[h]
                        return tl[:, (fq - _fa) * P:(fq - _fa + 1) * P]
                else:
                    w1q, w3q = [], []
                    for h in range(HT):
                        t1 = w13_pool.tile([P, FQ * P], BF16, tag="w13")
                        nc.sync.dma_start(
                            out=t1[:], in_=w1t_d[h * P:(h + 1) * P, f0:f0 + FQ * P])
                        w1q.append(t1)
                        t3 = w13_pool.tile([P, FQ * P], BF16, tag="w13")
                        nc.sync.dma_start(
                            out=t3[:], in_=w3t_d[h * P:(h + 1) * P, f0:f0 + FQ * P])
                        w3q.append(t3)

                    def lw(fq, h, which, _w1q=w1q, _w3q=w3q):
                        tl = (_w1q if which == 1 else _w3q)[h]
                        return tl[:, fq * P:(fq + 1) * P]
                w2q = []
                for fq in range(FQ):
                    f = q * FQ + fq
                    t2 = w2_pool.tile([P, H], BF16, tag="w2")
                    nc.sync.dma_start(out=t2[:], in_=w2t_d[f * P:(f + 1) * P, :])
                    w2q.append(t2)

                # ---- GEMM1/2: h1T/h3T [P(F), NW] + silu*mul -> g tiles
                # [P, TC]. Emitted h-major with p1/p3 interleaved per h so
                # compute tracks per-h DMA arrival during the ramp.
                gq = []
                for fq in range(FQ):
                    p1 = [ps12.tile([P, w], F32, tag="ps12", name=f"p1_{c}_{q}_{fq}_{th}")
                          for th, (o, w) in enumerate(nw_slices)]
                    p3 = [ps12.tile([P, w], F32, tag="ps12", name=f"p3_{c}_{q}_{fq}_{th}")
                          for th, (o, w) in enumerate(nw_slices)]
                    for h in range(HT):
                        l1 = lw(fq, h, 1)
                        for th, (o, w) in enumerate(nw_slices):
                            nc.tensor.matmul(
                                p1[th][:], l1, xts[h][:, o:o + w],
                                start=(h == 0), stop=(h == HT - 1))
                        l3 = lw(fq, h, 3)
                        for th, (o, w) in enumerate(nw_slices):
                            nc.tensor.matmul(
                                p3[th][:], l3, xts[h][:, o:o + w],
                                start=(h == 0), stop=(h == HT - 1))
                    gt = g_pool.tile([P, TC], BF16, tag="g")
                    for th, (o, w) in enumerate(nw_slices):
                        tmp = tmp_pool.tile([P, w], F32, tag="tmp")
                        if silu_native:
                            nc.scalar.activation(tmp[:], p1[th][:], AF.Silu)
                        else:
                            # CoreSim has no Silu; sigmoid then explicit mul
                            sg = tmp_pool.tile([P, w], F32, tag="tmp")
                            nc.scalar.activation(sg[:], p1[th][:], AF.Sigmoid)
                            nc.vector.tensor_tensor(tmp[:], sg[:], p1[th][:], OP.mult)
                        nc.vector.tensor_tensor(
                            gt[:, o:o + w], tmp[:], p3[th][:], OP.mult)
                    gq.append(gt)

                # ---- GEMM3 (transposed): out[H-part, tok-col] partial over
                # this quarter's F. w2 slice [128f, 128h] is the stationary,
                # g streams as the moving operand -- tokens cost their
                # natural count, not a 128-padded tile grid.
                for hg in range(HT):
                    if q == 0:
                        at = acc_pool.tile([P, TC], F32, tag="acc")
                        acc_tiles[hg] = at
                    else:
                        at = acc_tiles[hg]
                    for th, (o, w) in enumerate(nw_slices):
                        po = ps3.tile([P, w], F32, tag="ps3")
                        for fq in range(FQ):
                            nc.tensor.matmul(
                                po[:], w2q[fq][:, hg * P:(hg + 1) * P],
                                gq[fq][:, o:o + w],
                                start=(fq == 0), stop=(fq == FQ - 1))
                        if q == 0 and NQ > 1:
                            nc.scalar.copy(at[:, o:o + w], po[:])
                        elif q < NQ - 1:
                            nc.vector.tensor_tensor(
                                at[:, o:o + w], po[:], at[:, o:o + w], OP.add)
                        else:
                            ob = ob_pool.tile([P, w], BF16, tag="ob")
                            if NQ == 1:
                                nc.scalar.copy(ob[:], po[:])
                            else:
                                nc.vector.tensor_tensor(
                                    ob[:], po[:], at[:, o:o + w], OP.add)
                            nc.sync.dma_start(
                                out=out_d[hg * P:(hg + 1) * P,
                                          t0 + o:t0 + o + w],
                                in_=ob[:])

    nc.compile()
    return nc


def _routing(x2, gate_w):
    """Host replay of the router: token index list + renormalized top-2
    weight per (expert, token). Selection uses the same fp32 logits /
    stable argsort as the reference's top_k; weights computed in fp64."""
    logits = x2.astype(np.float32) @ gate_w.astype(np.float32).T
    order = np.argsort(-logits, axis=1, kind="stable")[:, :2]
    E = gate_w.shape[0]
    idx = [np.nonzero((order == e).any(axis=1))[0] for e in range(E)]
    l64 = logits.astype(np.float64)
    l64 -= l64.max(axis=1, keepdims=True)
    ex = np.exp(l64)
    p = ex / ex.sum(axis=1, keepdims=True)
    t = np.arange(p.shape[0])
    denom = p[t, order[:, 0]] + p[t, order[:, 1]]
    return idx, p, denom


def _host_top2_idx(x2, gate_w):
    """Token index list per expert (host copy of the routing, for sharding)."""
    return _routing(x2, gate_w)[0]


_NC_CACHE = {}


def _get_nc(key, **kw):
    if key not in _NC_CACHE:
        _NC_CACHE[key] = build_moe_nc(**kw)
    return _NC_CACHE[key]


def kernel(hidden_states, gate_w, w1, w2, w3, _trace=False, _trace_kwargs=None):
    B, S, H = hidden_states.shape
    E = gate_w.shape[0]
    T = B * S
    x2 = np.asarray(hidden_states, dtype=np.float32).reshape(T, H)
    idx, p, denom = _routing(x2, gate_w)
    tarange = np.arange(T)
    cmax = max(len(i) for i in idx)
    cpad = max(512, -(-cmax // 64) * 64)
    xt16 = np.ascontiguousarray(x2.T).astype(ml_dtypes.bfloat16)
    wdt = ml_dtypes.bfloat16

    def expert_weights(e):
        return {
            "w1t": np.ascontiguousarray(
                np.asarray(w1[e], dtype=np.float32).T).astype(wdt),
            "w3t": np.ascontiguousarray(
                np.asarray(w3[e], dtype=np.float32).T).astype(wdt),
            "w2t": np.ascontiguousarray(
                np.asarray(w2[e], dtype=np.float32).T).astype(wdt),
        }

    if cpad <= 2048:
        # sparse path: each core gets only its expert's tokens (padded)
        nc = _get_nc(("sparse", cpad), T=cpad, TC=cpad, NQ=4)
        in_maps = []
        for e in range(E):
            xg = np.zeros((H, cpad), dtype=ml_dtypes.bfloat16)
            xg[:, :len(idx[e])] = xt16[:, idx[e]]
            m = expert_weights(e)
            m["xt"] = xg
            in_maps.append(m)
        res = run_bass_kernel_spmd(
            nc, in_maps, list(range(E)), trace=_trace, **(_trace_kwargs or {}))
        kernel.last_results = res
        out = np.zeros((T, H), dtype=np.float32)
        for e, r in enumerate(res.results):
            n = len(idx[e])
            w_e = (p[idx[e], e] / denom[idx[e]]).astype(np.float32)
            out[idx[e]] += r["out"][:, :n].T.astype(np.float32) * w_e[:, None]
    else:
        # dense fallback (pathological routing imbalance): every core runs
        # all tokens for its expert; non-selected tokens get weight 0.
        nc = _get_nc(("dense", T), T=T, TC=1024, NQ=4)
        in_maps = []
        for e in range(E):
            m = expert_weights(e)
            m["xt"] = xt16
            in_maps.append(m)
        res = run_bass_kernel_spmd(
            nc, in_maps, list(range(E)), trace=_trace, **(_trace_kwargs or {}))
        kernel.last_results = res
        out = np.zeros((T, H), dtype=np.float32)
        for e, r in enumerate(res.results):
            w_e = np.zeros(T, dtype=np.float32)
            w_e[idx[e]] = (p[idx[e], e] / denom[idx[e]]).astype(np.float32)
            out += r["out"].T.astype(np.float32) * w_e[:, None]
    return out.reshape(B, S, H).astype(hidden_states.dtype)


# revision 24
# speedup vs baseline: 1.0182x; 1.0019x over previous
"""Mixtral MoE layer (top-2 of 8 experts) as a Trainium2 Bass/Tile kernel.

Strategy (expert-parallel, per the original tp_size/expert_indices code):
  - 8 NeuronCores, one expert per core. Sparse dispatch: the host replays the
    router (numpy) to decide token->core sharding AND to compute the final
    per-token routing weights; each core receives only the ~T/4 tokens routed
    to its expert (padded to a static 64-aligned capacity), its token scales,
    and its expert's weights. No routing runs on device at all.
  - Host pre-transposes x -> xT [H,C] and weights -> w1t/w3t [H,F], w2t [F,H]
    so every SBUF tile load is a natural contiguous 2D slice. x and weights
    are cast to bf16 on the host; all accumulation is fp32 in PSUM.
  - On device, each core runs its expert MLP in transposed space: h1T/h3T
    [F-part, tok-col] = w1t/w3t-tile.T @ xT, g = silu(h1T)*h3T (bf16),
    out [tok-part, H-col] = gT-tile.T @ w2t, then scales rows by the
    host-provided routing weight (a per-partition scalar) and writes a [C,H]
    partial. The host scatter-adds partials back to [T,H] ("all-reduce").
  - Startup is tuned against the PE HAM clock gate (1.2 -> 2.4 GHz): a short
    dummy-matmul warmup covers the ~6.5us framework preamble, x tiles stream
    on the scalar-engine HWDGE queue in parallel with weights on the sync
    queue, and the first quarter's w1/w3 loads are split into small head
    tiles so real GEMMs start as soon as data lands and the PE never idles
    (an idle window mid-ramp re-throttles the clock for ~40us).

Tiling: single token chunk of TC; F processed in NQ=4 quarters with an SBUF
fp32 accumulator for GEMM3 partials; moving-operand slices capped at 512;
token tiles of 128 with an optional 64 tail. Dense fallback (all tokens on
every core) guards pathological routing imbalance (capacity > 2048).
"""

from contextlib import ExitStack

import ml_dtypes
import numpy as np

import concourse.bacc as bacc
import concourse.tile as tile
from concourse import mybir
from concourse.bass_utils import run_bass_kernel_spmd

P = 128
AF = mybir.ActivationFunctionType
OP = mybir.AluOpType
AX = mybir.AxisListType
F32 = mybir.dt.float32
BF16 = mybir.dt.bfloat16


def build_moe_nc(T=1088, H=1024, F=3584, TC=1088, NQ=4, WU=4, silu_native=True):
    """Build the single-core SPMD program. Returns the compiled Bacc."""
    HT = H // P          # contraction k-tiles for GEMM1/2
    FT = F // P          # f tiles
    FQ = FT // NQ        # f-tiles per quarter
    NCH = T // TC        # token chunks
    # moving-operand slices of the token chunk (max free dim 512). Balanced
    # widths (e.g. 384/384/320 rather than 512/512/64): a tiny tail slice is
    # NX-issue-floor-bound (~56ns) while balanced slices stay stream-bound.
    nsl = -(-TC // 512)
    k64 = TC // 64
    nw_slices = []
    off = 0
    for i in range(nsl):
        w = (k64 // nsl + (1 if i < k64 % nsl else 0)) * 64
        nw_slices.append((off, w))
        off += w
    assert off == TC
    # 128-token tiles per chunk, with an optional 64-token tail tile
    ttiles = []
    toff = 0
    while toff < TC:
        th_ = min(P, TC - toff)
        ttiles.append((toff, th_))
        toff += th_
    NTT = len(ttiles)
    HW = min(512, H)     # output H column group width
    HC = H // HW
    NSC = NTT * NCH      # scale columns (one per token tile)
    FA = min(2, FQ)      # f-tiles in the quarter-0 "head" weight load
    assert FT % NQ == 0 and T % TC == 0 and TC % 64 == 0 and H % HW == 0

    nc = bacc.Bacc("TRN2", target_bir_lowering=False, debug=False)
    xt_d = nc.dram_tensor("xt", [H, T], BF16, kind="ExternalInput").ap()
    w1t_d = nc.dram_tensor("w1t", [H, F], BF16, kind="ExternalInput").ap()
    w3t_d = nc.dram_tensor("w3t", [H, F], BF16, kind="ExternalInput").ap()
    w2t_d = nc.dram_tensor("w2t", [F, H], BF16, kind="ExternalInput").ap()
    # transposed output [H, T]: GEMM3 runs with w2 stationary / g moving so
    # tokens stream at their natural count (no 128-padded tail tile); the
    # host applies the routing scale during the scatter-add.
    out_d = nc.dram_tensor("out", [H, T], BF16, kind="ExternalOutput").ap()

    with tile.TileContext(nc) as tc, ExitStack() as ctx:
        xt_pool = ctx.enter_context(tc.tile_pool(name="xt", bufs=HT))
        wa_pool = ctx.enter_context(tc.tile_pool(name="wa", bufs=2 * HT))
        wb_pool = ctx.enter_context(tc.tile_pool(name="wb", bufs=2 * HT))
        w13_pool = ctx.enter_context(tc.tile_pool(name="w13", bufs=2 * HT))
        w2_pool = ctx.enter_context(tc.tile_pool(name="w2", bufs=FQ))
        g_pool = ctx.enter_context(tc.tile_pool(name="g", bufs=FQ))
        acc_pool = ctx.enter_context(tc.tile_pool(name="acc", bufs=HT))
        tmp_pool = ctx.enter_context(tc.tile_pool(name="tmp", bufs=4))
        ob_pool = ctx.enter_context(tc.tile_pool(name="ob", bufs=4))
        ps12 = ctx.enter_context(tc.tile_pool(name="ps12", bufs=6, space="PSUM"))
        ps3 = ctx.enter_context(tc.tile_pool(name="ps3", bufs=2, space="PSUM"))

        # ---- PE warm-up: a few dummy matmuls bridging the gap between the
        # ~6.3us framework preamble barrier and the first x/w tile landing
        # (~9us), so the HAM clock-gate window starts counting early. Kept
        # short: the PE queue is FIFO, so an overshooting warmup delays the
        # first real GEMM past its data arrival.
        wu_w = tmp_pool.tile([P, P], BF16, tag="wu")
        wu_x = tmp_pool.tile([P, 512], BF16, tag="wu2")
        nc.vector.memset(wu_w[:], 0.0)
        nc.vector.memset(wu_x[:], 0.0)
        wu_ps = ps3.tile([P, 512], F32, tag="ps3")
        for i in range(WU):
            nc.tensor.matmul(wu_ps[:], wu_w[:], wu_x[:],
                             start=(i == 0), stop=(i == WU - 1))

        for c in range(NCH):
            t0 = c * TC
            # ---- xT chunk [H, TC] bf16 tiles; loads alternate between the
            # two HWDGE queues (scalar/sync) so triggers and transfers of
            # consecutive h-tiles overlap
            xts = []
            for h in range(HT):
                t_ = xt_pool.tile([P, TC], BF16, tag="xt")
                # sync queue's user stream starts ~3us before scalar's (act
                # tables load first there), so even trios go on sync
                eng = nc.sync if h % 2 == 0 else nc.scalar
                eng.dma_start(out=t_[:], in_=xt_d[h * P:(h + 1) * P, t0:t0 + TC])
                xts.append(t_)
                if c == 0:
                    # ramp: interleave the quarter-0 head weights per h on
                    # the same queue as x[h] so each h's (x, w1a, w3a) trio
                    # lands together, in consumption order
                    ta1 = wa_pool.tile([P, FA * P], BF16, tag="w1a")
                    eng.dma_start(out=ta1[:], in_=w1t_d[h * P:(h + 1) * P, 0:FA * P])
                    ta3 = wa_pool.tile([P, FA * P], BF16, tag="w3a")
                    eng.dma_start(out=ta3[:], in_=w3t_d[h * P:(h + 1) * P, 0:FA * P])
                    if h == 0:
                        w1h, w3h = [], []
                    w1h.append(ta1)
                    w3h.append(ta3)

            acc_tiles = {}
            for q in range(NQ):
                f0 = q * FQ * P
                # ---- quarter weight loads. Quarter 0 is split per h into a
                # small head (first FA f-tiles) + remainder so the very first
                # GEMMs are gated on ~65KB tiles, not a whole 1.8MB quarter.
                if q == 0 and c == 0:
                    w1r, w3r = [], []
                    for h in range(HT):
                        eng = nc.sync
                        tb1 = wb_pool.tile([P, (FQ - FA) * P], BF16, tag="w1b")
                        eng.dma_start(
                            out=tb1[:],
                            in_=w1t_d[h * P:(h + 1) * P, f0 + FA * P:f0 + FQ * P])
                        w1r.append(tb1)
                        tb3 = wb_pool.tile([P, (FQ - FA) * P], BF16, tag="w3b")
                        eng.dma_start(
                            out=tb3[:],
                            in_=w3t_d[h * P:(h + 1) * P, f0 + FA * P:f0 + FQ * P])
                        w3r.append(tb3)

                    def lw(fq, h, which,
                           _a1=w1h, _a3=w3h, _b1=w1r, _b3=w3r, _fa=FA):
                        if fq < _fa:
                            tl = (_a1 if which == 1 else _a3)[h]
                            return tl[:, fq * P:(fq + 1) * P]
                        tl = (_b1 if which == 1 else _b3)[h]
                        return tl[:, (fq - _fa) * P:(fq - _fa + 1) * P]
                else:
                    w1q, w3q = [], []
                    for h in range(HT):
                        t1 = w13_pool.tile([P, FQ * P], BF16, tag="w13")
                        nc.sync.dma_start(
                            out=t1[:], in_=w1t_d[h * P:(h + 1) * P, f0:f0 + FQ * P])
                        w1q.append(t1)
                        t3 = w13_pool.tile([P, FQ * P], BF16, tag="w13")
                        nc.sync.dma_start(
                            out=t3[:], in_=w3t_d[h * P:(h + 1) * P, f0:f0 + FQ * P])
                        w3q.append(t3)

                    def lw(fq, h, which, _w1q=w1q, _w3q=w3q):
                        tl = (_w1q if which == 1 else _w3q)[h]
                        return tl[:, fq * P:(fq + 1) * P]
                w2q = []
                for fq in range(FQ):
                    f = q * FQ + fq
                    t2 = w2_pool.tile([P, H], BF16, tag="w2")
                    nc.sync.dma_start(out=t2[:], in_=w2t_d[f * P:(f + 1) * P, :])
                    w2q.append(t2)

                # ---- GEMM1/2: h1T/h3T [P(F), NW] + silu*mul -> g tiles
                # [P, TC]. Emitted h-major with p1/p3 interleaved per h so
                # compute tracks per-h DMA arrival during the ramp.
                gq = []
                for fq in range(FQ):
                    p1 = [ps12.tile([P, w], F32, tag="ps12", name=f"p1_{c}_{q}_{fq}_{th}")
                          for th, (o, w) in enumerate(nw_slices)]
                    p3 = [ps12.tile([P, w], F32, tag="ps12", name=f"p3_{c}_{q}_{fq}_{th}")
                          for th, (o, w) in enumerate(nw_slices)]
                    for h in range(HT):
                        l1 = lw(fq, h, 1)
                        for th, (o, w) in enumerate(nw_slices):
                            nc.tensor.matmul(
                                p1[th][:], l1, xts[h][:, o:o + w],
                                start=(h == 0), stop=(h == HT - 1))
                        l3 = lw(fq, h, 3)
                        for th, (o, w) in enumerate(nw_slices):
                            nc.tensor.matmul(
                                p3[th][:], l3, xts[h][:, o:o + w],
                                start=(h == 0), stop=(h == HT - 1))
                    gt = g_pool.tile([P, TC], BF16, tag="g")
                    for th, (o, w) in enumerate(nw_slices):
                        tmp = tmp_pool.tile([P, w], F32, tag="tmp")
                        if silu_native:
                            nc.scalar.activation(tmp[:], p1[th][:], AF.Silu)
                        else:
                            # CoreSim has no Silu; sigmoid then explicit mul
                            sg = tmp_pool.tile([P, w], F32, tag="tmp")
                            nc.scalar.activation(sg[:], p1[th][:], AF.Sigmoid)
                            nc.vector.tensor_tensor(tmp[:], sg[:], p1[th][:], OP.mult)
                        nc.vector.tensor_tensor(
                            gt[:, o:o + w], tmp[:], p3[th][:], OP.mult)
                    gq.append(gt)

                # ---- GEMM3 (transposed): out[H-part, tok-col] partial over
                # this quarter's F. w2 slice [128f, 128h] is the stationary,
                # g streams as the moving operand -- tokens cost their
                # natural count, not a 128-padded tile grid.
                for hg in range(HT):
                    if q == 0:
                        at = acc_pool.tile([P, TC], F32, tag="acc")
                        acc_tiles[hg] = at
                    else:
                        at = acc_tiles[hg]
                    for th, (o, w) in enumerate(nw_slices):
                        po = ps3.tile([P, w], F32, tag="ps3")
                        for fq in range(FQ):
                            nc.tensor.matmul(
                                po[:], w2q[fq][:, hg * P:(hg + 1) * P],
                                gq[fq][:, o:o + w],
                                start=(fq == 0), stop=(fq == FQ - 1))
                        if q == 0 and NQ > 1:
                            nc.scalar.copy(at[:, o:o + w], po[:])
                        elif q < NQ - 1:
                            nc.vector.tensor_tensor(
                                at[:, o:o + w], po[:], at[:, o:o + w], OP.add)
                        else:
                            ob = ob_pool.tile([P, w], BF16, tag="ob")
                            if NQ == 1:
                                nc.scalar.copy(ob[:], po[:])
                            else:
                                nc.vector.tensor_tensor(
                                    ob[:], po[:], at[:, o:o + w], OP.add)
                            nc.sync.dma_start(
                                out=out_d[hg * P:(hg + 1) * P,
                                          t0 + o:t0 + o + w],
                                in_=ob[:])

    nc.compile()
    return nc


def _routing(x2, gate_w):
    """Host replay of the router: token index list + renormalized top-2
    weight per (expert, token). Selection uses the same fp32 logits /
    stable argsort as the reference's top_k; weights computed in fp64."""
    logits = x2.astype(np.float32) @ gate_w.astype(np.float32).T
    order = np.argsort(-logits, axis=1, kind="stable")[:, :2]
    E = gate_w.shape[0]
    idx = [np.nonzero((order == e).any(axis=1))[0] for e in range(E)]
    l64 = logits.astype(np.float64)
    l64 -= l64.max(axis=1, keepdims=True)
    ex = np.exp(l64)
    p = ex / ex.sum(axis=1, keepdims=True)
    t = np.arange(p.shape[0])
    denom = p[t, order[:, 0]] + p[t, order[:, 1]]
    return idx, p, denom


def _host_top2_idx(x2, gate_w):
    """Token index list per expert (host copy of the routing, for sharding)."""
    return _routing(x2, gate_w)[0]


_NC_CACHE = {}


def _get_nc(key, **kw):
    if key not in _NC_CACHE:
        _NC_CACHE[key] = build_moe_nc(**kw)
    return _NC_CACHE[key]


def kernel(hidden_states, gate_w, w1, w2, w3, _trace=False, _trace_kwargs=None):
    B, S, H = hidden_states.shape
    E = gate_w.shape[0]
    T = B * S
    x2 = np.asarray(hidden_states, dtype=np.float32).reshape(T, H)
    idx, p, denom = _routing(x2, gate_w)
    tarange = np.arange(T)
    cmax = max(len(i) for i in idx)
    cpad = max(512, -(-cmax // 64) * 64)
    xt16 = np.ascontiguousarray(x2.T).astype(ml_dtypes.bfloat16)
    wdt = ml_dtypes.bfloat16

    def expert_weights(e):
        return {
            "w1t": np.ascontiguousarray(
                np.asarray(w1[e], dtype=np.float32).T).astype(wdt),
            "w3t": np.ascontiguousarray(
                np.asarray(w3[e], dtype=np.float32).T).astype(wdt),
            "w2t": np.ascontiguousarray(
                np.asarray(w2[e], dtype=np.float32).T).astype(wdt),
        }

    if cpad <= 2048:
        # sparse path: each core gets only its expert's tokens (padded)
        nc = _get_nc(("sparse", cpad), T=cpad, TC=cpad, NQ=4)
        in_maps = []
        for e in range(E):
            xg = np.zeros((H, cpad), dtype=ml_dtypes.bfloat16)
            xg[:, :len(idx[e])] = xt16[:, idx[e]]
            m = expert_weights(e)
            m["xt"] = xg
            in_maps.append(m)
        res = run_bass_kernel_spmd(
            nc, in_maps, list(range(E)), trace=_trace, **(_trace_kwargs or {}))
        kernel.last_results = res
        out = np.zeros((T, H), dtype=np.float32)
        for e, r in enumerate(res.results):
            n = len(idx[e])
            w_e = (p[idx[e], e] / denom[idx[e]]).astype(np.float32)
            out[idx[e]] += r["out"][:, :n].T.astype(np.float32) * w_e[:, None]
    else:
        # dense fallback (pathological routing imbalance): every core runs
        # all tokens for its expert; non-selected tokens get weight 0.
        nc = _get_nc(("dense", T), T=T, TC=1024, NQ=4)
        in_maps = []
        for e in range(E):
            m = expert_weights(e)
            m["xt"] = xt16
            in_maps.append(m)
        res = run_bass_kernel_spmd(
            nc, in_maps, list(range(E)), trace=_trace, **(_trace_kwargs or {}))
        kernel.last_results = res
        out = np.zeros((T, H), dtype=np.float32)
        for e, r in enumerate(res.results):
            w_e = np.zeros(T, dtype=np.float32)
            w_e[idx[e]] = (p[idx[e], e] / denom[idx[e]]).astype(np.float32)
            out += r["out"].T.astype(np.float32) * w_e[:, None]
    return out.reshape(B, S, H).astype(hidden_states.dtype)


# revision 28
# speedup vs baseline: 1.0473x; 1.0286x over previous
"""Mixtral MoE layer (top-2 of 8 experts) as a Trainium2 Bass/Tile kernel.

Strategy (expert-parallel, per the original tp_size/expert_indices code):
  - 8 NeuronCores, one expert per core. Sparse dispatch: the host replays the
    router (numpy) to decide token->core sharding AND to compute the final
    per-token routing weights; each core receives only the ~T/4 tokens routed
    to its expert (padded to a static 64-aligned capacity), its token scales,
    and its expert's weights. No routing runs on device at all.
  - Host pre-transposes x -> xT [H,C] and weights -> w1t/w3t [H,F], w2t [F,H]
    so every SBUF tile load is a natural contiguous 2D slice. x and weights
    are cast to bf16 on the host; all accumulation is fp32 in PSUM.
  - On device, each core runs its expert MLP in transposed space: h1T/h3T
    [F-part, tok-col] = w1t/w3t-tile.T @ xT, g = silu(h1T)*h3T (bf16),
    out [tok-part, H-col] = gT-tile.T @ w2t, then scales rows by the
    host-provided routing weight (a per-partition scalar) and writes a [C,H]
    partial. The host scatter-adds partials back to [T,H] ("all-reduce").
  - Startup is tuned against the PE HAM clock gate (1.2 -> 2.4 GHz): a short
    dummy-matmul warmup covers the ~6.5us framework preamble, x tiles stream
    on the scalar-engine HWDGE queue in parallel with weights on the sync
    queue, and the first quarter's w1/w3 loads are split into small head
    tiles so real GEMMs start as soon as data lands and the PE never idles
    (an idle window mid-ramp re-throttles the clock for ~40us).

Tiling: single token chunk of TC; F processed in NQ=4 quarters with an SBUF
fp32 accumulator for GEMM3 partials; moving-operand slices capped at 512;
token tiles of 128 with an optional 64 tail. Dense fallback (all tokens on
every core) guards pathological routing imbalance (capacity > 2048).
"""

from contextlib import ExitStack

import ml_dtypes
import numpy as np

import concourse.bacc as bacc
import concourse.tile as tile
from concourse import mybir
from concourse.bass_utils import run_bass_kernel_spmd

P = 128
AF = mybir.ActivationFunctionType
OP = mybir.AluOpType
AX = mybir.AxisListType
F32 = mybir.dt.float32
BF16 = mybir.dt.bfloat16


def build_moe_nc(T=1088, H=1024, F=3584, TC=1088, NQ=4, WU=4, silu_native=True):
    """Build the single-core SPMD program. Returns the compiled Bacc."""
    HT = H // P          # contraction k-tiles for GEMM1/2
    FT = F // P          # f tiles
    FQ = FT // NQ        # f-tiles per quarter
    NCH = T // TC        # token chunks
    # moving-operand slices of the token chunk (max free dim 512). Balanced
    # widths (e.g. 384/384/320 rather than 512/512/64): a tiny tail slice is
    # NX-issue-floor-bound (~56ns) while balanced slices stay stream-bound.
    nsl = -(-TC // 512)
    k64 = TC // 64
    nw_slices = []
    off = 0
    for i in range(nsl):
        w = (k64 // nsl + (1 if i < k64 % nsl else 0)) * 64
        nw_slices.append((off, w))
        off += w
    assert off == TC
    # 128-token tiles per chunk, with an optional 64-token tail tile
    ttiles = []
    toff = 0
    while toff < TC:
        th_ = min(P, TC - toff)
        ttiles.append((toff, th_))
        toff += th_
    NTT = len(ttiles)
    HW = min(512, H)     # output H column group width
    HC = H // HW
    NSC = NTT * NCH      # scale columns (one per token tile)
    FA = min(2, FQ)      # f-tiles in the quarter-0 "head" weight load
    assert FT % NQ == 0 and T % TC == 0 and TC % 64 == 0 and H % HW == 0

    nc = bacc.Bacc("TRN2", target_bir_lowering=False, debug=False)
    xt_d = nc.dram_tensor("xt", [H, T], BF16, kind="ExternalInput").ap()
    w1t_d = nc.dram_tensor("w1t", [H, F], BF16, kind="ExternalInput").ap()
    w3t_d = nc.dram_tensor("w3t", [H, F], BF16, kind="ExternalInput").ap()
    w2t_d = nc.dram_tensor("w2t", [F, H], BF16, kind="ExternalInput").ap()
    # transposed output [H, T]: GEMM3 runs with w2 stationary / g moving so
    # tokens stream at their natural count (no 128-padded tail tile); the
    # host applies the routing scale during the scatter-add.
    out_d = nc.dram_tensor("out", [H, T], BF16, kind="ExternalOutput").ap()

    with tile.TileContext(nc) as tc, ExitStack() as ctx:
        xt_pool = ctx.enter_context(tc.tile_pool(name="xt", bufs=HT))
        wa_pool = ctx.enter_context(tc.tile_pool(name="wa", bufs=2 * HT))
        wb_pool = ctx.enter_context(tc.tile_pool(name="wb", bufs=2 * HT))
        w13_pool = ctx.enter_context(tc.tile_pool(name="w13", bufs=2 * HT))
        w2_pool = ctx.enter_context(tc.tile_pool(name="w2", bufs=FQ))
        g_pool = ctx.enter_context(tc.tile_pool(name="g", bufs=FQ))
        acc_pool = ctx.enter_context(tc.tile_pool(name="acc", bufs=HT))
        tmp_pool = ctx.enter_context(tc.tile_pool(name="tmp", bufs=4))
        ob_pool = ctx.enter_context(tc.tile_pool(name="ob", bufs=4))
        ps12 = ctx.enter_context(tc.tile_pool(name="ps12", bufs=6, space="PSUM"))
        ps3 = ctx.enter_context(tc.tile_pool(name="ps3", bufs=2, space="PSUM"))

        # ---- PE warm-up: a few dummy matmuls bridging the gap between the
        # ~6.3us framework preamble barrier and the first x/w tile landing
        # (~9us), so the HAM clock-gate window starts counting early. Kept
        # short: the PE queue is FIFO, so an overshooting warmup delays the
        # first real GEMM past its data arrival.
        wu_w = tmp_pool.tile([P, P], BF16, tag="wu")
        wu_x = tmp_pool.tile([P, 512], BF16, tag="wu2")
        nc.vector.memset(wu_w[:], 0.0)
        nc.vector.memset(wu_x[:], 0.0)
        wu_ps = ps3.tile([P, 512], F32, tag="ps3")
        for i in range(WU):
            nc.tensor.matmul(wu_ps[:], wu_w[:], wu_x[:],
                             start=(i == 0), stop=(i == WU - 1))

        for c in range(NCH):
            t0 = c * TC
            # ---- xT chunk [H, TC] bf16 tiles; loads alternate between the
            # two HWDGE queues (scalar/sync) so triggers and transfers of
            # consecutive h-tiles overlap
            xts = []
            for h in range(HT):
                t_ = xt_pool.tile([P, TC], BF16, tag="xt")
                # sync queue's user stream starts ~3us before scalar's (act
                # tables load first there), so even trios go on sync
                eng = nc.sync if h % 2 == 0 else nc.scalar
                eng.dma_start(out=t_[:], in_=xt_d[h * P:(h + 1) * P, t0:t0 + TC])
                xts.append(t_)
                if c == 0:
                    # ramp: interleave the quarter-0 head weights per h on
                    # the same queue as x[h] so each h's (x, w1a, w3a) trio
                    # lands together, in consumption order
                    ta1 = wa_pool.tile([P, FA * P], BF16, tag="w1a")
                    eng.dma_start(out=ta1[:], in_=w1t_d[h * P:(h + 1) * P, 0:FA * P])
                    ta3 = wa_pool.tile([P, FA * P], BF16, tag="w3a")
                    eng.dma_start(out=ta3[:], in_=w3t_d[h * P:(h + 1) * P, 0:FA * P])
                    if h == 0:
                        w1h, w3h = [], []
                    w1h.append(ta1)
                    w3h.append(ta3)

            acc_tiles = {}
            for q in range(NQ):
                f0 = q * FQ * P
                # ---- quarter weight loads. Quarter 0 is split per h into a
                # small head (first FA f-tiles) + remainder so the very first
                # GEMMs are gated on ~65KB tiles, not a whole 1.8MB quarter.
                if q == 0 and c == 0:
                    w1r, w3r = [], []
                    for h in range(HT):
                        eng = nc.sync
                        tb1 = wb_pool.tile([P, (FQ - FA) * P], BF16, tag="w1b")
                        eng.dma_start(
                            out=tb1[:],
                            in_=w1t_d[h * P:(h + 1) * P, f0 + FA * P:f0 + FQ * P])
                        w1r.append(tb1)
                        tb3 = wb_pool.tile([P, (FQ - FA) * P], BF16, tag="w3b")
                        eng.dma_start(
                            out=tb3[:],
                            in_=w3t_d[h * P:(h + 1) * P, f0 + FA * P:f0 + FQ * P])
                        w3r.append(tb3)

                    def lw(fq, h, which,
                           _a1=w1h, _a3=w3h, _b1=w1r, _b3=w3r, _fa=FA):
                        if fq < _fa:
                            tl = (_a1 if which == 1 else _a3)[h]
                            return tl[:, fq * P:(fq + 1) * P]
                        tl = (_b1 if which == 1 else _b3)[h]
                        return tl[:, (fq - _fa) * P:(fq - _fa + 1) * P]
                else:
                    w1q, w3q = [], []
                    for h in range(HT):
                        t1 = w13_pool.tile([P, FQ * P], BF16, tag="w13")
                        nc.sync.dma_start(
                            out=t1[:], in_=w1t_d[h * P:(h + 1) * P, f0:f0 + FQ * P])
                        w1q.append(t1)
                        t3 = w13_pool.tile([P, FQ * P], BF16, tag="w13")
                        nc.sync.dma_start(
                            out=t3[:], in_=w3t_d[h * P:(h + 1) * P, f0:f0 + FQ * P])
                        w3q.append(t3)

                    def lw(fq, h, which, _w1q=w1q, _w3q=w3q):
                        tl = (_w1q if which == 1 else _w3q)[h]
                        return tl[:, fq * P:(fq + 1) * P]
                w2q = []
                for fq in range(FQ):
                    f = q * FQ + fq
                    t2 = w2_pool.tile([P, H], BF16, tag="w2")
                    nc.sync.dma_start(out=t2[:], in_=w2t_d[f * P:(f + 1) * P, :])
                    w2q.append(t2)

                # ---- GEMM1/2: h1T/h3T [P(F), NW] + silu*mul -> g tiles
                # [P, TC]. Emitted h-major with p1/p3 interleaved per h so
                # compute tracks per-h DMA arrival during the ramp.
                gq = []
                for fq in range(FQ):
                    p1 = [ps12.tile([P, w], F32, tag="ps12", name=f"p1_{c}_{q}_{fq}_{th}")
                          for th, (o, w) in enumerate(nw_slices)]
                    p3 = [ps12.tile([P, w], F32, tag="ps12", name=f"p3_{c}_{q}_{fq}_{th}")
                          for th, (o, w) in enumerate(nw_slices)]
                    for h in range(HT):
                        l1 = lw(fq, h, 1)
                        for th, (o, w) in enumerate(nw_slices):
                            nc.tensor.matmul(
                                p1[th][:], l1, xts[h][:, o:o + w],
                                start=(h == 0), stop=(h == HT - 1))
                        l3 = lw(fq, h, 3)
                        for th, (o, w) in enumerate(nw_slices):
                            nc.tensor.matmul(
                                p3[th][:], l3, xts[h][:, o:o + w],
                                start=(h == 0), stop=(h == HT - 1))
                    gt = g_pool.tile([P, TC], BF16, tag="g")
                    for th, (o, w) in enumerate(nw_slices):
                        tmp = tmp_pool.tile([P, w], F32, tag="tmp")
                        if silu_native:
                            nc.scalar.activation(tmp[:], p1[th][:], AF.Silu)
                        else:
                            # CoreSim has no Silu; sigmoid then explicit mul
                            sg = tmp_pool.tile([P, w], F32, tag="tmp")
                            nc.scalar.activation(sg[:], p1[th][:], AF.Sigmoid)
                            nc.vector.tensor_tensor(tmp[:], sg[:], p1[th][:], OP.mult)
                        nc.vector.tensor_tensor(
                            gt[:, o:o + w], tmp[:], p3[th][:], OP.mult)
                    gq.append(gt)

                # ---- GEMM3 (transposed): out[H-part, tok-col] partial over
                # this quarter's F. w2 slice [128f, 128h] is the stationary,
                # g streams as the moving operand -- tokens cost their
                # natural count, not a 128-padded tile grid.
                for hg in range(HT):
                    if q == 0:
                        at = acc_pool.tile([P, TC], F32, tag="acc")
                        acc_tiles[hg] = at
                    else:
                        at = acc_tiles[hg]
                    for th, (o, w) in enumerate(nw_slices):
                        po = ps3.tile([P, w], F32, tag="ps3")
                        for fq in range(FQ):
                            nc.tensor.matmul(
                                po[:], w2q[fq][:, hg * P:(hg + 1) * P],
                                gq[fq][:, o:o + w],
                                start=(fq == 0), stop=(fq == FQ - 1))
                        if q == 0 and NQ > 1:
                            nc.scalar.copy(at[:, o:o + w], po[:])
                        elif q < NQ - 1:
                            nc.vector.tensor_tensor(
                                at[:, o:o + w], po[:], at[:, o:o + w], OP.add)
                        else:
                            ob = ob_pool.tile([P, w], BF16, tag="ob")
                            if NQ == 1:
                                nc.scalar.copy(ob[:], po[:])
                            else:
                                nc.vector.tensor_tensor(
                                    ob[:], po[:], at[:, o:o + w], OP.add)
                            nc.sync.dma_start(
                                out=out_d[hg * P:(hg + 1) * P,
                                          t0 + o:t0 + o + w],
                                in_=ob[:])

    nc.compile()
    return nc


def _routing(x2, gate_w):
    """Host replay of the router: token index list + renormalized top-2
    weight per (expert, token). Selection uses the same fp32 logits /
    stable argsort as the reference's top_k; weights computed in fp64."""
    logits = x2.astype(np.float32) @ gate_w.astype(np.float32).T
    order = np.argsort(-logits, axis=1, kind="stable")[:, :2]
    E = gate_w.shape[0]
    idx = [np.nonzero((order == e).any(axis=1))[0] for e in range(E)]
    l64 = logits.astype(np.float64)
    l64 -= l64.max(axis=1, keepdims=True)
    ex = np.exp(l64)
    p = ex / ex.sum(axis=1, keepdims=True)
    t = np.arange(p.shape[0])
    denom = p[t, order[:, 0]] + p[t, order[:, 1]]
    return idx, p, denom


def _host_top2_idx(x2, gate_w):
    """Token index list per expert (host copy of the routing, for sharding)."""
    return _routing(x2, gate_w)[0]


def build_moe_pair_nc(capA=1088, capB=1024, H=1024, Fh=1792, NQ=2, WU=4, FA=2,
                      silu_native=True):
    """Paired-expert program: each core runs TWO token segments (the pair's
    heavy expert, then its light expert) over HALF of F; the sibling core
    runs the other F-half and the host adds the two partials. Pairing a
    heavy expert with a light one cuts the per-core padded capacity from
    2*max_e(C_e) to max_heavy + max_light slots per core pair."""
    HT = H // P
    FT = Fh // P
    FQ = FT // NQ
    TT = capA + capB
    assert FT % NQ == 0 and capA % 64 == 0 and capB % 64 == 0

    nc = bacc.Bacc("TRN2", target_bir_lowering=False, debug=False)
    xt_d = nc.dram_tensor("xt", [H, TT], BF16, kind="ExternalInput").ap()
    wd = {}
    for sfx in ("A", "B"):
        wd["w1" + sfx] = nc.dram_tensor(
            "w1t" + sfx, [H, Fh], BF16, kind="ExternalInput").ap()
        wd["w3" + sfx] = nc.dram_tensor(
            "w3t" + sfx, [H, Fh], BF16, kind="ExternalInput").ap()
        wd["w2" + sfx] = nc.dram_tensor(
            "w2t" + sfx, [Fh, H], BF16, kind="ExternalInput").ap()
    out_d = nc.dram_tensor("out", [H, TT], BF16, kind="ExternalOutput").ap()

    with tile.TileContext(nc) as tc, ExitStack() as ctx:
        xt_pool = ctx.enter_context(tc.tile_pool(name="xt", bufs=HT))
        wa_pool = ctx.enter_context(tc.tile_pool(name="wa", bufs=2 * HT))
        wb_pool = ctx.enter_context(tc.tile_pool(name="wb", bufs=2 * HT))
        w13_pool = ctx.enter_context(tc.tile_pool(name="w13", bufs=2 * HT))
        w2_pool = ctx.enter_context(tc.tile_pool(name="w2", bufs=FQ))
        g_pool = ctx.enter_context(tc.tile_pool(name="g", bufs=FQ))
        acc_pool = ctx.enter_context(tc.tile_pool(name="acc", bufs=HT))
        tmp_pool = ctx.enter_context(tc.tile_pool(name="tmp", bufs=4))
        ob_pool = ctx.enter_context(tc.tile_pool(name="ob", bufs=4))
        ps12 = ctx.enter_context(tc.tile_pool(name="ps12", bufs=6, space="PSUM"))
        ps3 = ctx.enter_context(tc.tile_pool(name="ps3", bufs=2, space="PSUM"))

        wu_w = tmp_pool.tile([P, P], BF16, tag="wu")
        wu_x = tmp_pool.tile([P, 512], BF16, tag="wu2")
        nc.vector.memset(wu_w[:], 0.0)
        nc.vector.memset(wu_x[:], 0.0)
        wu_ps = ps3.tile([P, 512], F32, tag="ps3")
        for i in range(WU):
            nc.tensor.matmul(wu_ps[:], wu_w[:], wu_x[:],
                             start=(i == 0), stop=(i == WU - 1))

        t0 = 0
        for si, (sfx, TC) in enumerate((("A", capA), ("B", capB))):
            w1t_d, w3t_d, w2t_d = wd["w1" + sfx], wd["w3" + sfx], wd["w2" + sfx]
            nsl = -(-TC // 512)
            k64 = TC // 64
            nw_slices = []
            off = 0
            for i in range(nsl):
                w = (k64 // nsl + (1 if i < k64 % nsl else 0)) * 64
                nw_slices.append((off, w))
                off += w
            assert off == TC

            xts = []
            w1h, w3h = [], []
            for h in range(HT):
                t_ = xt_pool.tile([P, TC], BF16, tag="xt" + sfx)
                eng = (nc.sync if h % 2 == 0 else nc.scalar) if si == 0 else nc.sync
                eng.dma_start(out=t_[:], in_=xt_d[h * P:(h + 1) * P, t0:t0 + TC])
                xts.append(t_)
                if si == 0:
                    ta1 = wa_pool.tile([P, FA * P], BF16, tag="w1a")
                    eng.dma_start(out=ta1[:], in_=w1t_d[h * P:(h + 1) * P, 0:FA * P])
                    w1h.append(ta1)
                    ta3 = wa_pool.tile([P, FA * P], BF16, tag="w3a")
                    eng.dma_start(out=ta3[:], in_=w3t_d[h * P:(h + 1) * P, 0:FA * P])
                    w3h.append(ta3)

            acc_tiles = {}
            for q in range(NQ):
                f0 = q * FQ * P
                if q == 0 and si == 0:
                    w1r, w3r = [], []
                    for h in range(HT):
                        tb1 = wb_pool.tile([P, (FQ - FA) * P], BF16, tag="w1b")
                        nc.sync.dma_start(
                            out=tb1[:],
                            in_=w1t_d[h * P:(h + 1) * P, f0 + FA * P:f0 + FQ * P])
                        w1r.append(tb1)
                        tb3 = wb_pool.tile([P, (FQ - FA) * P], BF16, tag="w3b")
                        nc.sync.dma_start(
                            out=tb3[:],
                            in_=w3t_d[h * P:(h + 1) * P, f0 + FA * P:f0 + FQ * P])
                        w3r.append(tb3)

                    def lw(fq, h, which,
                           _a1=w1h, _a3=w3h, _b1=w1r, _b3=w3r, _fa=FA):
                        if fq < _fa:
                            tl = (_a1 if which == 1 else _a3)[h]
                            return tl[:, fq * P:(fq + 1) * P]
                        tl = (_b1 if which == 1 else _b3)[h]
                        return tl[:, (fq - _fa) * P:(fq - _fa + 1) * P]
                else:
                    w1q, w3q = [], []
                    for h in range(HT):
                        t1 = w13_pool.tile([P, FQ * P], BF16, tag="w13")
                        nc.sync.dma_start(
                            out=t1[:], in_=w1t_d[h * P:(h + 1) * P, f0:f0 + FQ * P])
                        w1q.append(t1)
                        t3 = w13_pool.tile([P, FQ * P], BF16, tag="w13")
                        nc.sync.dma_start(
                            out=t3[:], in_=w3t_d[h * P:(h + 1) * P, f0:f0 + FQ * P])
                        w3q.append(t3)

                    def lw(fq, h, which, _w1q=w1q, _w3q=w3q):
                        tl = (_w1q if which == 1 else _w3q)[h]
                        return tl[:, fq * P:(fq + 1) * P]
                w2q = []
                for fq in range(FQ):
                    f = q * FQ + fq
                    t2 = w2_pool.tile([P, H], BF16, tag="w2")
                    nc.sync.dma_start(out=t2[:], in_=w2t_d[f * P:(f + 1) * P, :])
                    w2q.append(t2)

                gq = []
                for fq in range(FQ):
                    p1 = [ps12.tile([P, w], F32, tag="ps12",
                                    name=f"p1_{si}_{q}_{fq}_{th}")
                          for th, (o, w) in enumerate(nw_slices)]
                    p3 = [ps12.tile([P, w], F32, tag="ps12",
                                    name=f"p3_{si}_{q}_{fq}_{th}")
                          for th, (o, w) in enumerate(nw_slices)]
                    for h in range(HT):
                        l1 = lw(fq, h, 1)
                        for th, (o, w) in enumerate(nw_slices):
                            nc.tensor.matmul(
                                p1[th][:], l1, xts[h][:, o:o + w],
                                start=(h == 0), stop=(h == HT - 1))
                        l3 = lw(fq, h, 3)
                        for th, (o, w) in enumerate(nw_slices):
                            nc.tensor.matmul(
                                p3[th][:], l3, xts[h][:, o:o + w],
                                start=(h == 0), stop=(h == HT - 1))
                    gt = g_pool.tile([P, TC], BF16, tag="g")
                    for th, (o, w) in enumerate(nw_slices):
                        tmp = tmp_pool.tile([P, w], F32, tag="tmp")
                        if silu_native:
                            nc.scalar.activation(tmp[:], p1[th][:], AF.Silu)
                        else:
                            sg = tmp_pool.tile([P, w], F32, tag="tmp")
                            nc.scalar.activation(sg[:], p1[th][:], AF.Sigmoid)
                            nc.vector.tensor_tensor(tmp[:], sg[:], p1[th][:], OP.mult)
                        nc.vector.tensor_tensor(
                            gt[:, o:o + w], tmp[:], p3[th][:], OP.mult)
                    gq.append(gt)

                for hg in range(HT):
                    if q == 0:
                        at = acc_pool.tile([P, TC], F32, tag="acc",
                                           name=f"acc_{si}_{hg}")
                        acc_tiles[hg] = at
                    else:
                        at = acc_tiles[hg]
                    for th, (o, w) in enumerate(nw_slices):
                        po = ps3.tile([P, w], F32, tag="ps3")
                        for fq in range(FQ):
                            nc.tensor.matmul(
                                po[:], w2q[fq][:, hg * P:(hg + 1) * P],
                                gq[fq][:, o:o + w],
                                start=(fq == 0), stop=(fq == FQ - 1))
                        if q == 0 and NQ > 1:
                            nc.scalar.copy(at[:, o:o + w], po[:])
                        elif q < NQ - 1:
                            nc.vector.tensor_tensor(
                                at[:, o:o + w], po[:], at[:, o:o + w], OP.add)
                        else:
                            ob = ob_pool.tile([P, w], BF16, tag="ob")
                            if NQ == 1:
                                nc.scalar.copy(ob[:], po[:])
                            else:
                                nc.vector.tensor_tensor(
                                    ob[:], po[:], at[:, o:o + w], OP.add)
                            nc.sync.dma_start(
                                out=out_d[hg * P:(hg + 1) * P,
                                          t0 + o:t0 + o + w],
                                in_=ob[:])
            t0 += TC

    nc.compile()
    return nc


_NC_CACHE = {}


def _get_nc(key, builder=build_moe_nc, **kw):
    if key not in _NC_CACHE:
        _NC_CACHE[key] = builder(**kw)
    return _NC_CACHE[key]


def kernel(hidden_states, gate_w, w1, w2, w3, _trace=False, _trace_kwargs=None):
    B, S, H = hidden_states.shape
    E = gate_w.shape[0]
    T = B * S
    x2 = np.asarray(hidden_states, dtype=np.float32).reshape(T, H)
    idx, p, denom = _routing(x2, gate_w)
    tarange = np.arange(T)
    cmax = max(len(i) for i in idx)
    cpad = max(512, -(-cmax // 64) * 64)
    xt16 = np.ascontiguousarray(x2.T).astype(ml_dtypes.bfloat16)
    wdt = ml_dtypes.bfloat16

    def expert_weights(e):
        return {
            "w1t": np.ascontiguousarray(
                np.asarray(w1[e], dtype=np.float32).T).astype(wdt),
            "w3t": np.ascontiguousarray(
                np.asarray(w3[e], dtype=np.float32).T).astype(wdt),
            "w2t": np.ascontiguousarray(
                np.asarray(w2[e], dtype=np.float32).T).astype(wdt),
        }

    F = w1.shape[1]
    Fh = F // 2

    def cvt(a):
        return np.ascontiguousarray(np.asarray(a, dtype=np.float32).T).astype(
            ml_dtypes.bfloat16)

    if cpad <= 2048 and E % 2 == 0 and Fh % P == 0:
        # paired-expert path: heavy+light experts share a core pair, each
        # core taking half of F for both; host adds the two partials
        loads = np.array([len(i) for i in idx])
        order_e = np.argsort(-loads, kind="stable")
        pairs = [(int(order_e[k]), int(order_e[E - 1 - k]))
                 for k in range(E // 2)]
        capA = max(512, -(-int(max(loads[eb] for eb, _ in pairs)) // 64) * 64)
        capB = max(512, -(-int(max(loads[es] for _, es in pairs)) // 64) * 64)
        nc = _get_nc(("pair", capA, capB), builder=build_moe_pair_nc,
                     capA=capA, capB=capB, H=H, Fh=Fh)
        in_maps = []
        for k, (eb, es) in enumerate(pairs):
            xg = np.zeros((H, capA + capB), dtype=ml_dtypes.bfloat16)
            xg[:, :loads[eb]] = xt16[:, idx[eb]]
            xg[:, capA:capA + loads[es]] = xt16[:, idx[es]]
            for half in range(2):
                lo, hi = half * Fh, (half + 1) * Fh
                in_maps.append({
                    "xt": xg,
                    "w1tA": cvt(w1[eb][lo:hi]),
                    "w3tA": cvt(w3[eb][lo:hi]),
                    "w2tA": cvt(np.asarray(w2[eb])[:, lo:hi]),
                    "w1tB": cvt(w1[es][lo:hi]),
                    "w3tB": cvt(w3[es][lo:hi]),
                    "w2tB": cvt(np.asarray(w2[es])[:, lo:hi]),
                })
        res = run_bass_kernel_spmd(
            nc, in_maps, list(range(E)), trace=_trace, **(_trace_kwargs or {}))
        kernel.last_results = res
        out = np.zeros((T, H), dtype=np.float32)
        for k, (eb, es) in enumerate(pairs):
            r0 = res.results[2 * k]["out"]
            r1 = res.results[2 * k + 1]["out"]
            for e, base in ((eb, 0), (es, capA)):
                n = len(idx[e])
                w_e = (p[idx[e], e] / denom[idx[e]]).astype(np.float32)
                s = (r0[:, base:base + n].astype(np.float32)
                     + r1[:, base:base + n].astype(np.float32)).T
                out[idx[e]] += s * w_e[:, None]
    elif cpad <= 2048:
        # sparse path: each core gets only its expert's tokens (padded)
        nc = _get_nc(("sparse", cpad), T=cpad, TC=cpad, NQ=4)
        in_maps = []
        for e in range(E):
            xg = np.zeros((H, cpad), dtype=ml_dtypes.bfloat16)
            xg[:, :len(idx[e])] = xt16[:, idx[e]]
            m = expert_weights(e)
            m["xt"] = xg
            in_maps.append(m)
        res = run_bass_kernel_spmd(
            nc, in_maps, list(range(E)), trace=_trace, **(_trace_kwargs or {}))
        kernel.last_results = res
        out = np.zeros((T, H), dtype=np.float32)
        for e, r in enumerate(res.results):
            n = len(idx[e])
            w_e = (p[idx[e], e] / denom[idx[e]]).astype(np.float32)
            out[idx[e]] += r["out"][:, :n].T.astype(np.float32) * w_e[:, None]
    else:
        # dense fallback (pathological routing imbalance): every core runs
        # all tokens for its expert; non-selected tokens get weight 0.
        nc = _get_nc(("dense", T), T=T, TC=1024, NQ=4)
        in_maps = []
        for e in range(E):
            m = expert_weights(e)
            m["xt"] = xt16
            in_maps.append(m)
        res = run_bass_kernel_spmd(
            nc, in_maps, list(range(E)), trace=_trace, **(_trace_kwargs or {}))
        kernel.last_results = res
        out = np.zeros((T, H), dtype=np.float32)
        for e, r in enumerate(res.results):
            w_e = np.zeros(T, dtype=np.float32)
            w_e[idx[e]] = (p[idx[e], e] / denom[idx[e]]).astype(np.float32)
            out += r["out"].T.astype(np.float32) * w_e[:, None]
    return out.reshape(B, S, H).astype(hidden_states.dtype)
